# revision 2
# baseline (speedup 1.0000x reference)
"""DiracScheduler kernel v4: baseline with an 8-partitions-per-row argmax.

Identical to the baseline except pos is loaded as [128, 512] per wave
(partition 8e+oct holds eighth oct of row e), halving the column-bound DVE
MAX8/FIND_INDEX8 passes (~0.65us each instead of ~1.22) so wave-1 shifts are
ready ~2us earlier and wave-2 ~3.5us earlier. The per-wave argmax-combine is
the same exact min-select chain with an 8-way (instead of 4-way) group.
m8/i8/if32/gm are double-buffered per wave since both waves now span all 128
partitions. The row-copy machinery is byte-identical to the baseline.
"""
from contextlib import ExitStack

import numpy as np

import concourse.bass as bass
import concourse.bacc as bacc
import concourse.mybir as mybir
from concourse import bass_utils

B = 8  # batch == n_cores

N = 65536
S = 4096
E = 32
UP = N // S  # 16
OCT = 8
CSo = S // OCT  # 512
LARGE = 65536.0
EH = E // 2  # 16 rows per wave

# per-engine rows: (wave1 slice, wave2 slice) of each wave's 16 rows
WAVE_ROWS = {
    "sync": (list(range(0, 6)), list(range(16, 22))),
    "scalar": (list(range(6, 12)), list(range(22, 28))),
    "gpsimd": (list(range(12, 16)), list(range(28, 32))),
}
N_HW_ROWS = 24
N_GP_ROWS = 8


def _build_core_program(nc):
    f32, u32 = mybir.dt.float32, mybir.dt.uint32
    f = nc.dram_tensor("f", [E * 2 * N], f32, kind="ExternalInput")
    pos = nc.dram_tensor("pos", [E, S], f32, kind="ExternalInput")
    out = nc.dram_tensor("out", [E, N], f32, kind="ExternalOutput")
    f_ap, out_ap, pos_ap = f.ap(), out.ap(), pos.ap()

    alu = mybir.AluOpType
    X = mybir.AxisListType.X

    # wave h, half k: rows 16h+8k .. +8 spread over 64 partitions
    pos_q = {}
    for h in range(2):
        for k in range(2):
            r0 = 16 * h + 8 * k
            pos_q[(h, k)] = pos_ap[r0 : r0 + 8, :].rearrange(
                "e (o c) -> (e o) c", o=OCT
            )

    with ExitStack() as ctx:
        sb = lambda name, shape, dt: ctx.enter_context(nc.sbuf_tensor(name, shape, dt))
        ps = lambda name, shape, dt: ctx.enter_context(nc.psum_tensor(name, shape, dt))
        sem = lambda name: ctx.enter_context(nc.semaphore(name))
        pos_sb = [sb(f"pos_sb{h}", [128, CSo], f32) for h in range(2)]
        m8 = [sb(f"m8_{h}", [128, 8], f32) for h in range(2)]
        i8 = [sb(f"i8_{h}", [128, 8], u32) for h in range(2)]
        if32 = [sb(f"if32_{h}", [128, 1], f32) for h in range(2)]
        ident = sb("ident", [128, 128], f32)
        qoff_row = sb("qoff_row", [1, 128], f32)
        g_row = sb("g_row", [1, 128], f32)
        gm_row = [sb(f"gm_row{h}", [1, 128], f32) for h in range(2)]
        vbest = sb("vbest", [1, E], f32)
        mask_row = sb("mask_row", [1, 128], u32)
        gfin = sb("gfin", [1, E], f32)
        t16_row = sb("t16_row", [1, E], u32)
        pm = [ps("pm1", [1, 128], f32), ps("pm2", [1, 128], f32)]
        pi = [ps("pi1", [1, 128], f32), ps("pi2", [1, 128], f32)]
        sem_pos1 = sem("sem_pos1")
        sem_pos2 = sem("sem_pos2")
        sem_pos3 = sem("sem_pos3")
        sem_pos4 = sem("sem_pos4")
        sem_gp = sem("sem_gp")
        sem_v = sem("sem_v")
        sem_pe = sem("sem_pe")
        sem_ready1 = sem("sem_ready1")
        sem_ready2 = sem("sem_ready2")
        sem_dma = sem("sem_dma")
        sem_dma_gp = sem("sem_dma_gp")
        block = ctx.enter_context(nc.Block())

        sem_pos = {(0, 0): sem_pos1, (0, 1): sem_pos2,
                   (1, 0): sem_pos3, (1, 1): sem_pos4}
        sem_ready = [sem_ready1, sem_ready2]
        vcount = [0]
        m_marks = [0, 0]
        c_marks = [0, 0]

        def vinc(inst, target_sem=None):
            if target_sem is None:
                vcount[0] += 1
                inst.then_inc(sem_v, 1)
            else:
                inst.then_inc(target_sem, 1)
            return inst

        def dve_wave(vector, h):
            """Emit one wave's DVE chain over all 128 partitions."""
            elo = h * EH
            vector.wait_ge(sem_pos[(h, 0)], 16)
            vector.wait_ge(sem_pos[(h, 1)], 16)
            vinc(vector.max(out=m8[h][:, :], in_=pos_sb[h][:, :]))
            m_marks[h] = m_done = vcount[0]
            vector.wait_ge(sem_v, m_done)
            vinc(vector.max_index(i8[h][:, :], m8[h][:, :], pos_sb[h][:, :]))
            vector.wait_ge(sem_v, vcount[0])
            vinc(vector.tensor_copy(if32[h][:, :], i8[h][:, 0:1]))
            c_marks[h] = vcount[0]
            vector.wait_ge(sem_pe, 2 * (h + 1))  # pm[h], pi[h] done
            vector.wait_ge(sem_gp, 3)            # qoff ready
            pm3 = pm[h].ap().rearrange("p (e o) -> p e o", o=OCT)
            vinc(
                vector.tensor_tensor(
                    g_row[0:1, :], pi[h].ap()[:], qoff_row[0:1, :],
                    op=alu.add,
                )
            )
            vinc(
                vector.tensor_reduce(
                    vbest[0:1, elo : elo + EH], pm3, axis=X, op=alu.max
                )
            )
            vector.wait_ge(sem_v, vcount[0])
            vb_b = (
                vbest[0:1, elo : elo + EH]
                .rearrange("p (e o) -> p e o", o=1)
                .to_broadcast([1, EH, OCT])
            )
            vinc(
                vector.tensor_tensor(
                    mask_row[0:1, :].rearrange("p (e o) -> p e o", o=OCT),
                    pm3, vb_b, op=alu.is_equal,
                )
            )
            vector.wait_ge(sem_v, vcount[0])
            vector.wait_ge(sem_gp, 4)  # gm rows prefilled with LARGE
            vinc(
                vector.copy_predicated(
                    gm_row[h][0:1, :], mask_row[0:1, :], g_row[0:1, :],
                )
            )
            vector.wait_ge(sem_v, vcount[0])
            vinc(
                vector.tensor_reduce(
                    gfin[0:1, elo : elo + EH],
                    gm_row[h][0:1, :].rearrange("p (e o) -> p e o", o=OCT),
                    axis=X, op=alu.min,
                )
            )
            vector.wait_ge(sem_v, vcount[0])
            vector.tensor_scalar(
                t16_row[0:1, elo : elo + EH], gfin[0:1, elo : elo + EH],
                float(UP), scalar2=None, op0=alu.mult,
            ).then_inc(sem_ready[h], 1)

        def dma_rows(engine, rows, dsem, wave):
            engine.wait_ge(sem_ready[wave], 1)
            regs = [engine.alloc_register(f"off{e}") for e in rows]
            engine.load(regs[0:1], t16_row[0:1, rows[0] : rows[0] + 1])
            for k, e in enumerate(rows):
                engine.reg_alu(regs[k], (2 * e + 1) * N, regs[k], alu.subtract)
                off = engine.snap(
                    regs[k], donate=True, min_val=UP, max_val=(2 * e + 1) * N
                )
                engine.dma_start(out_ap[e, :], f_ap[bass.ds(off, N)]).then_inc(
                    dsem, 16
                )
                if k == 0 and len(rows) > 1:
                    engine.load(
                        regs[1:], t16_row[0:1, rows[0] + 1 : rows[0] + len(rows)]
                    )

        def dma_tail(engine):
            engine.wait_ge(sem_dma, N_HW_ROWS * 16)
            engine.wait_ge(sem_dma_gp, N_GP_ROWS * 16)

        @block.gpsimd
        def _(gpsimd):
            gpsimd.memset(ident[:], 0.0).then_inc(sem_gp, 1)
            gpsimd.wait_ge(sem_gp, 1)
            gpsimd.affine_select(
                out=ident[:], in_=ident[:], compare_op=alu.not_equal,
                fill=1.0, base=0, pattern=[[-1, 128]], channel_multiplier=1,
            ).then_inc(sem_gp, 1)
            for o in range(OCT):
                ms = gpsimd.memset(
                    qoff_row[:].rearrange("p (e o) -> p o e", o=OCT)[0:1, o, :],
                    float(CSo * o),
                )
            ms.then_inc(sem_gp, 1)
            gpsimd.memset(gm_row[0][:], LARGE)
            gpsimd.memset(gm_row[1][:], LARGE).then_inc(sem_gp, 1)  # -> 4
            dma_rows(gpsimd, WAVE_ROWS["gpsimd"][0], sem_dma_gp, 0)
            dma_rows(gpsimd, WAVE_ROWS["gpsimd"][1], sem_dma_gp, 1)
            dma_tail(gpsimd)

        @block.vector
        def _(vector):
            dve_wave(vector, 0)
            dve_wave(vector, 1)

        @block.tensor
        def _(tensor):
            tensor.wait_ge(sem_gp, 2)
            for h in range(2):
                tensor.wait_ge(sem_v, m_marks[h])
                nc.tensor.transpose(
                    pm[h].ap()[:], m8[h][:, 0:1], ident[:, :]
                ).then_inc(sem_pe, 1)
                tensor.wait_ge(sem_v, c_marks[h])
                nc.tensor.transpose(
                    pi[h].ap()[:], if32[h][:, :], ident[:, :]
                ).then_inc(sem_pe, 1)

        @block.sync
        def _(sync):
            sync.dma_start(pos_sb[0][0:64, :], pos_q[(0, 0)]).then_inc(sem_pos1, 16)
            sync.dma_start(pos_sb[1][0:64, :], pos_q[(1, 0)]).then_inc(sem_pos3, 16)
            dma_rows(sync, WAVE_ROWS["sync"][0], sem_dma, 0)
            dma_rows(sync, WAVE_ROWS["sync"][1], sem_dma, 1)
            dma_tail(sync)

        @block.scalar
        def _(scalar):
            scalar.dma_start(pos_sb[0][64:128, :], pos_q[(0, 1)]).then_inc(sem_pos2, 16)
            scalar.dma_start(pos_sb[1][64:128, :], pos_q[(1, 1)]).then_inc(sem_pos4, 16)
            dma_rows(scalar, WAVE_ROWS["scalar"][0], sem_dma, 0)
            dma_rows(scalar, WAVE_ROWS["scalar"][1], sem_dma, 1)
            dma_tail(scalar)

    return nc


LAST_RESULTS = None  # BassKernelResults of the most recent run (for profiling)
_NC = None


def _get_nc():
    global _NC
    if _NC is None:
        nc = bacc.Bacc(
            "TRN2",
            target_bir_lowering=False,
            debug=False,
            enable_asserts=False,
            num_devices=B,
        )
        _build_core_program(nc)
        nc.compile()
        _NC = nc
    return _NC


def kernel(events: np.ndarray, pos: np.ndarray) -> np.ndarray:
    global LAST_RESULTS
    nc = _get_nc()

    events = np.ascontiguousarray(events, dtype=np.float32)
    pos_2d = np.ascontiguousarray(np.asarray(pos).reshape(E, S), dtype=np.float32)

    in_maps = []
    for b in range(B):
        F = np.zeros((E, 2, N), np.float32)
        F[:, 1, :] = events[b]
        in_maps.append({"f": F.reshape(-1), "pos": pos_2d})

    res = bass_utils.run_bass_kernel_spmd(nc, in_maps, core_ids=list(range(B)))
    LAST_RESULTS = res
    return np.stack([res.results[b]["out"] for b in range(B)], axis=0)
